# revision 2
# baseline (speedup 1.0000x reference)
"""Distributed brute-force KNN (IndexFlatL2, K=3) + mean of gathered pred values.

v4 over the baseline: startup/tail surgery around the fixed 169us fp8 matmul
floor (800 DoubleRow matmuls at 211ns; LdWeights fully hidden; PE at the
documented 157 TF/s peak).
  - One HWDGE queue sustains only ~180 B/ns, so the startup working set is
    spread across sync + scalar HWDGE queues and the gpsimd SWDGE queue:
    qt arrives as per-bc chunks that group 1's bc-loop consumes as they land,
    and the first mov group is split in half across both HWDGE queues.
  - mT is pre-blocked on host into per-group contiguous slabs (one fat
    4-20KB line per partition per group).
  - 8B priming transfers absorb the ~1.5us DGE queue spin-up.
  - Warmup matmuls bridge preamble -> data-ready with no PE gap: any idle
    demotes the HAM clock to half speed for ~7us (the dominant baseline
    startup loss).
  - Groups [1,2,4,5,5,4,3,1]: small groups while DMA ramps, small last group
    to shrink the post-matmul drain; wt output DMAs alternate queues.
Device math identical to baseline: fp8e4m3 DoubleRow matmuls compute
s[b,n] = 2q.m_n - ||m_n||^2 via 4 digit rows; DVE window-max (10) -> DRAM.
Host: rank 8*1250 windows/query, exact-rescore top WSEL windows, top-3, mean.
"""

import sys
import types

import ml_dtypes
import numpy as np

try:
    import antenv.axon_hooks  # noqa: F401
except ImportError:
    _stub = types.ModuleType("antenv.axon_hooks")
    _stub.get_axon_ntff_profile_hook = lambda: None
    _stub.set_axon_ntff_profile_hook = lambda hook: None
    sys.modules["antenv.axon_hooks"] = _stub

import concourse.bacc as bacc
import concourse.mybir as mybir
import concourse.tile as tile
from concourse import bass_utils

B = 1024
D = 1024
N = 100000
NCORES = 8
NS = N // NCORES    # 12500
BLK = 500
NBLK = NS // BLK    # 25
KT = D // 128       # 8
BCH = B // 128      # 8
WND = 10
NWIN = NS // WND    # 1250
WPB = BLK // WND    # 50
K = 3
WSEL = 64
GROUPS = [(0, 1), (1, 2), (3, 4), (7, 5), (12, 5), (17, 4), (21, 3), (24, 1)]
WARMUP_BIG = 11     # 500-col warmups bridge to data-ready with no PE idle
WARMUP_SM = 0

_CACHE = {}
LAST_RUN = None
LAST_TOP_IDX = None


def _build_program():
    nc = bacc.Bacc(
        "TRN2",
        target_bir_lowering=False,
        debug=False,
        enable_asserts=False,
        num_devices=NCORES,
    )
    f32 = mybir.dt.float32
    fp8 = mybir.dt.float8e4

    # group-blocked: per partition p the layout is [g][o][n'] contiguous
    mT = nc.dram_tensor("mT", [128, KT * NS], fp8, kind="ExternalInput").ap()
    # [p, bc, o, b'] contiguous
    qT = nc.dram_tensor("qT", [128, BCH * KT * 128], fp8, kind="ExternalInput").ap()
    # group-major output: [p][g][c][j'] so each group is one fat-line DMA
    out_w = nc.dram_tensor(
        "out_w", [128, BCH * NWIN], f32, kind="ExternalOutput").ap()

    qT_r = qT.rearrange("p (c o b) -> p c o b", c=BCH, o=KT)

    with tile.TileContext(nc) as tc:
        with (
            tc.tile_pool(name="const", bufs=1) as cpool,
            tc.tile_pool(name="mov", bufs=3) as movpool,
            tc.tile_pool(name="wm", bufs=4) as wmpool,
            tc.tile_pool(name="psum", bufs=8, space="PSUM") as pspool,
        ):
            # 8B priming transfers to spin up both HWDGE queues
            prime = cpool.tile([1, 8], fp8, tag="prime")
            nc.sync.dma_start(prime, mT[:1, :8])
            prime2 = cpool.tile([1, 8], fp8, tag="prime2")
            nc.scalar.dma_start(prime2, mT[:1, 8:16])

            qt_sb = cpool.tile([128, BCH * KT * 128], fp8, tag="qt")

            # warmup weights: memset on gpsimd (off the DMA queues)
            wu_q = cpool.tile([128, 2, 128], fp8, tag="wuq")
            wu_m = cpool.tile([128, 2, BLK], fp8, tag="wum")
            nc.gpsimd.memset(wu_q, 0.0)
            nc.gpsimd.memset(wu_m, 0.0)

            # Aggregate DMA bandwidth (~330 B/ns) is shared across queues,
            # so only the startup-critical set moves first: mov_g1 (scalar)
            # and qt per-bc chunks (sync) sized so group 1's bc-loop rides
            # the arrival cadence.  Everything else staggers in behind on
            # sync/scalar/gpsimd with deadlines far in the future.
            # HWDGE queues move ~25 packets/us (one packet per partition
            # line), SWDGE ~2x that.  So: qt as ONE 128-packet DMA on the
            # SWDGE queue; the critical first mov group split BY PARTITION
            # across the two HWDGE queues (64 packets each ~ 2.6us).
            # baseline-like startup (best measured luck profile): qt first
            # on sync (8KB lines), first mov group on scalar, later groups
            # alternate scalar/gpsimd so no queue develops a backlog
            nc.sync.dma_start(qt_sb, qT)
            movs = []
            off = 0
            for gi, (g0, w) in enumerate(GROUPS):
                wn = w * BLK
                mov = movpool.tile([128, KT * wn], fp8, tag="mov",
                                   name=f"mov{gi}")
                movs.append(mov)
                src = mT[:, off: off + KT * wn]
                if gi in (0, 2, 4):         # scalar: g1, g3, g5
                    nc.scalar.dma_start(mov, src)
                elif gi in (1, 3, 5):       # gpsimd: g2, g4, g6
                    nc.gpsimd.dma_start(mov, src)
                else:                       # sync: g7, g8
                    nc.sync.dma_start(mov, src)
                off += KT * wn

            qt_r4 = qt_sb.rearrange("p (c o b) -> p c o b", c=BCH, o=KT)

            wu_ps = pspool.tile([128, BLK], f32, tag="mm", name="mm_ps")
            for i in range(WARMUP_BIG):
                nc.tensor.matmul(
                    wu_ps,
                    lhsT=wu_q,
                    rhs=wu_m,
                    start=True,
                    stop=True,
                    perf_mode=mybir.MatmulPerfMode.DoubleRow,
                )
            for i in range(WARMUP_SM):
                nc.tensor.matmul(
                    wu_ps[:, :64],
                    lhsT=wu_q,
                    rhs=wu_m[:, :, :64],
                    start=True,
                    stop=True,
                    perf_mode=mybir.MatmulPerfMode.DoubleRow,
                )

            obase = 0
            for gi, (g0, w) in enumerate(GROUPS):
                mov = movs[gi].rearrange("p (o n) -> p o n", o=KT)
                wt = wmpool.tile([128, BCH * w * WPB], f32, tag="wt",
                                 name=f"wt{gi}")
                wt_r = wt.rearrange("p (c j) -> p c j", c=BCH)
                for bc in range(BCH):
                    for j in range(w):
                        ps = pspool.tile([128, BLK], f32, tag="mm", name="mm_ps")
                        for k in range(0, KT, 2):
                            nc.tensor.matmul(
                                ps,
                                lhsT=qt_r4[:, bc, k: k + 2, :],
                                rhs=mov[:, k: k + 2, j * BLK: (j + 1) * BLK],
                                start=(k == 0),
                                stop=(k + 2 >= KT),
                                perf_mode=mybir.MatmulPerfMode.DoubleRow,
                            )
                        nc.vector.tensor_reduce(
                            wt_r[:, bc, j * WPB: (j + 1) * WPB],
                            ps.rearrange("p (w t) -> p w t", t=WND),
                            axis=mybir.AxisListType.X,
                            op=mybir.AluOpType.max,
                            opt_input=False,
                        )
                # one fat-line DMA per group: [p][c][j'] packed, contiguous
                sz = BCH * w * WPB
                outq = nc.sync if gi % 2 == 0 else nc.scalar
                outq.dma_start(out_w[:, obase: obase + sz], wt)
                obase += sz
    nc.compile()
    return nc


def kernel(h_query, memory_embeds, pred_values):
    global LAST_RUN, LAST_TOP_IDX
    q = np.ascontiguousarray(np.asarray(h_query, dtype=np.float32))
    m = np.ascontiguousarray(np.asarray(memory_embeds, dtype=np.float32))
    pv = np.asarray(pred_values, dtype=np.float32)

    # -||m||^2 folded into the contraction as 4 digit rows
    msq = np.einsum("nd,nd->n", m.astype(np.float64), m.astype(np.float64))
    a = np.rint(msq / 128.0)
    r = msq - 128.0 * a
    b = np.rint(r / 16.0)
    r -= 16.0 * b
    c = np.rint(r / 2.0)
    r -= 2.0 * c
    d = np.rint(r / 0.25)
    digit_rows = np.stack([-a, -b, -c, -d]).astype(np.float32)  # [4, N]

    fp8 = ml_dtypes.float8_e4m3
    qTs = np.empty((D, B), dtype=fp8)
    qTs[: D - 4] = (q.T[: D - 4] * np.float32(2.0)).astype(fp8)
    qTs[D - 4:] = np.array([128.0, 16.0, 2.0, 0.25], dtype=np.float32)[
        :, None
    ].astype(fp8)
    # [o*128+p, bc*128+b'] -> [p, bc, o, b'] contiguous
    qTp = np.ascontiguousarray(
        qTs.reshape(KT, 128, BCH, 128).transpose(1, 2, 0, 3).reshape(128, -1)
    )
    mTs = np.empty((D, N), dtype=fp8)
    mTs[: D - 4] = m.T[: D - 4].astype(fp8)
    mTs[D - 4:] = digit_rows.astype(fp8)

    if "nc" not in _CACHE:
        _CACHE["nc"] = _build_program()
    nc = _CACHE["nc"]

    in_maps = []
    for cix in range(NCORES):
        sl = slice(cix * NS, (cix + 1) * NS)
        mc = np.ascontiguousarray(mTs[:, sl]).reshape(KT, 128, NS)
        mc = mc.transpose(1, 0, 2)  # [p, o, n]
        slabs = [
            np.ascontiguousarray(mc[:, :, g0 * BLK: (g0 + w) * BLK]).reshape(128, -1)
            for (g0, w) in GROUPS
        ]
        mty = np.ascontiguousarray(np.concatenate(slabs, axis=1))
        in_maps.append({"mT": mty, "qT": qTp})

    res = bass_utils.run_bass_kernel_spmd(nc, in_maps, core_ids=list(range(NCORES)))
    LAST_RUN = res
    results = res.results

    wall = np.empty((B, NCORES * NWIN), dtype=np.float32)
    for cix, r in enumerate(results):
        ow = r["out_w"]  # [128, sum_g 8*w*WPB], [p][g][c][j'] packed
        obase = 0
        for (g0, w) in GROUPS:
            seg = ow[:, obase: obase + BCH * w * WPB].reshape(
                128, BCH, w * WPB)
            obase += BCH * w * WPB
            # query b = c*128 + p ; window col = cix*NWIN + g0*WPB + j'
            wall[:, cix * NWIN + g0 * WPB: cix * NWIN + (g0 + w) * WPB] = (
                seg.transpose(1, 0, 2).reshape(B, w * WPB)
            )

    sel = np.argpartition(-wall, WSEL, axis=1)[:, :WSEL]      # [B, WSEL]
    core = sel // NWIN
    rows = (core * NS + (sel % NWIN) * WND)[:, :, None] + np.arange(WND)[
        None, None, :
    ]
    cidx = rows.reshape(B, WSEL * WND)

    q64 = q.astype(np.float64)
    m64 = m.astype(np.float64)
    msq64 = msq
    top_idx = np.empty((B, K), dtype=np.int64)
    CH = 128
    for b0 in range(0, B, CH):
        ci = cidx[b0: b0 + CH]
        mg = m64[ci]
        s = 2.0 * np.einsum("bd,bkd->bk", q64[b0: b0 + CH], mg)
        s -= msq64[ci]
        pick = np.argpartition(-s, K, axis=1)[:, :K]
        top_idx[b0: b0 + CH] = np.take_along_axis(ci, pick, axis=1)
    LAST_TOP_IDX = top_idx
    y = pv[top_idx].astype(np.float64).mean()
    return np.float32(y)
